# revision 76
# baseline (speedup 1.0000x reference)
"""Trainium2 Bass kernel for nn_Attention_40785009443452.

Reference computation (per batch b):
    qkv = w_qkv @ x_b            # 1x1x1 conv == channel linear
    q,k,v split into 4 heads of dim 16, tokens N = 16*16*16 = 4096
    q,k L2-normalized along head dim
    attn = softmax(q @ k^T)      # [N, N] per (b, head)
    out  = attn @ v  (+ x residual)

Sharding: 8 (batch, head) pairs -> 8 NeuronCores (data + head parallel).
Each core computes one full 4096x4096 attention.

Device algorithm (per core), S^T orientation so softmax reduction (over
keys) lands on the PSUM partition axis and is folded into the PV matmul
via an appended ones-column on V:

    B  = Wq^T Wk                     [64, 64]   (tiny matmul on device)
    G  = B^T X                       [64, 4096]
    G' = G * rq  (column scale)      rq[n] = 1/||q_n||
    S^T tile [128 keys, 1024 qry] = X_j^T(chunk) @ G'(cols)   K=64 matmul
    P^T = exp(rk[m] * S^T)           rk on ACT per-partition scale
    O' [17, 1024] += V'_j^T @ P^T    V' = [V_j | ones]  -> row 16 = denom
    out^T = O'[0:16] / O'[16] + x_res

All normalization scales are computed as exp(-0.5*ln(sumsq)) on ScalarE
(Rsqrt/Reciprocal activations are banned for accuracy; Ln+Exp live in one
ACT table set so there are no table switches).
"""

import numpy as np

import concourse.bass as bass
import concourse.mybir as mybir
import concourse.tile as tile
from concourse import bacc
from concourse.bass_utils import run_bass_kernel_spmd

NCORES = 8
C = 64          # channels
HEADS = 4
HD = 16         # head dim
N = 4096        # tokens (16*16*16)
NBQ = 1024      # queries per outer block
NB = N // NBQ   # 4 outer blocks
KC = 128        # keys per chunk
JT = N // KC    # 32 key chunks
FP = mybir.dt.float32

# dtype for the P = exp(S) tiles and V' (the PV matmul operands)
PT_DT = mybir.dt.bfloat16
# dtype for the S^T matmul operands (X stationary copy + G' moving)
S_DT = mybir.dt.bfloat16

AF = mybir.ActivationFunctionType


def build_program():
    nc = bacc.Bacc(
        "TRN2", target_bir_lowering=False, debug=False, enable_asserts=False
    )
    x_d = nc.dram_tensor("x", [C, N], FP, kind="ExternalInput").ap()
    w_d = nc.dram_tensor("w", [3 * HD, C], FP, kind="ExternalInput").ap()
    wT_d = nc.dram_tensor("wT", [C, 3 * HD], FP, kind="ExternalInput").ap()
    xr_d = nc.dram_tensor("xres", [HD, N], FP, kind="ExternalInput").ap()
    op_d = nc.dram_tensor("onespat", [2 * HD, 33], FP,
                          kind="ExternalInput").ap()
    out_d = nc.dram_tensor("out", [HD, N], FP, kind="ExternalOutput").ap()
    scr_d = nc.dram_tensor("rk_scratch", [1, N], FP, kind="Internal").ap()

    with tile.TileContext(nc) as tc:
        _body(tc, x_d, w_d, wT_d, xr_d, op_d, out_d, scr_d)
    nc.compile()
    return nc


def _body(tc, x_d, w_d, wT_d, xr_d, op_d, out_d, scr_d):
    nc = tc.nc
    import contextlib

    import os

    # Pre-load the one ACT table set that contains Exp, Ln AND Square, so the
    # compiler's per-function chooser doesn't flip-flop between
    # exp_and_others and natural_log (35 table loads = ~45us of ACT time).
    if os.environ.get("K_PRELOAD", "1") == "1":
        from concourse.hw_specs import get_activation_tables

        set_names = list(get_activation_tables(nc.m.arch).keys())
        set_id = set_names.index("natural_log_exp_and_others")
        nc.scalar.add_instruction(
            mybir.InstLoadActFuncSet(
                name=f"I-{nc.next_id()}", act_func_set_id=set_id
            )
        )

    with contextlib.ExitStack() as ctx:
        consts = ctx.enter_context(tc.tile_pool(name="consts", bufs=1))

        # ---- load inputs -------------------------------------------------
        # weights on a separate DMA queue (small, needed first); x chunks on
        # the sync queue so compute starts as soon as chunk 0 lands.
        wq_eng = nc.gpsimd if os.environ.get("K_GPDMA", "1") == "1" else nc.sync
        Wq = consts.tile([HD, C], FP)
        wq_eng.dma_start(Wq, w_d[0:HD, :])
        Wk = consts.tile([HD, C], FP)
        wq_eng.dma_start(Wk, w_d[HD : 2 * HD, :])
        WT = consts.tile([C, 3 * HD], FP)
        wq_eng.dma_start(WT, wT_d)
        X = consts.tile([C, N], FP)
        for c8 in range(8):
            sl = slice(c8 * 512, c8 * 512 + 512)
            nc.sync.dma_start(X[:, sl], x_d[:, sl])
        XR = consts.tile([HD, N], FP)
        wq_eng.dma_start(XR, xr_d)

        ones1_16 = consts.tile([1, HD], S_DT)
        nc.any.memset(ones1_16, 1.0)
        eps_b = consts.tile([KC, 1], FP)
        nc.any.memset(eps_b, 1e-24)

        # Duplicated-row (both halves identical) bf16 operands: the two
        # 512-column S matmuls of each key chunk run on PE row groups 0-63
        # and 64-127 — alternating row groups lets the PE pull LDWEIGHTS
        # ahead and run the K=64 matmuls concurrently (2.2x measured).
        Bsb2 = consts.tile([C, 2 * C], S_DT)   # [B | B] stationary
        Gp2 = consts.tile([2 * C, N], S_DT)    # G'*rq duplicated rows
        Xs2 = consts.tile([2 * C, N], S_DT)    # X duplicated rows
        Xp2 = consts.tile([2 * C, N], S_DT)    # X*rk duplicated rows
        WTb = consts.tile([C, 2 * HD], S_DT)   # [Wq^T | Wk^T] in bf16
        ones_pat_f = consts.tile([2 * HD, 33], FP)
        nc.sync.dma_start(ones_pat_f, op_d)
        ones_pat = consts.tile([2 * HD, 33], S_DT)
        nc.vector.tensor_copy(ones_pat, ones_pat_f)
        ones1_128 = consts.tile([1, 2 * C], S_DT)
        nc.any.memset(ones1_128, 1.0)
        # [V_j(16) | zeros(16) | ones(1)] stationary tiles; the ones column
        # lands the softmax denominator on PSUM partition 32 (32-aligned
        # reads are a BIR verifier requirement).
        Vp = consts.tile([KC, JT, 33], PT_DT)

        nc.any.memset(Vp, 0.0)
        nc.any.memset(Vp[:, :, 32], 1.0)

        with contextlib.ExitStack() as mctx:
            pps = mctx.enter_context(
                tc.tile_pool(name="prol_ps", bufs=8, space="PSUM"))
            psb = mctx.enter_context(tc.tile_pool(name="prol_sb", bufs=6))

            # B = Wq^T Wk (tiny, fp32), duplicated into [B | B] bf16
            ps_b = pps.tile([C, C], FP, tag="pp")
            nc.tensor.matmul(ps_b, Wq, Wk, start=True, stop=True)
            nc.vector.tensor_copy(Bsb2[:, 0:C], ps_b)
            nc.vector.tensor_copy(Bsb2[:, C : 2 * C], ps_b)
            nc.vector.tensor_copy(WTb, WT[:, 0 : 2 * HD])

            # Per 512-column chunk: q norms + k norms/V tiles (interleaved so
            # PE always has independent work while ACT runs Ln/Exp).
            for c8 in range(8):
                sl = slice(c8 * 512, c8 * 512 + 512)
                nc.vector.tensor_copy(Xs2[0:C, sl], X[:, sl])
                nc.vector.tensor_copy(Xs2[C : 2 * C, sl], X[:, sl])
                # q + k norms (orientation 1): sumsq_q -> partition 0,
                # sumsq_k -> partition 32 of ps_nq
                ps_q = pps.tile([2 * HD, 512], FP, tag="pp")
                nc.tensor.matmul(ps_q, WTb, Xs2[0:C, sl],
                                 start=True, stop=True)
                sqq = psb.tile([2 * HD, 512], S_DT, tag="sq")
                nc.scalar.activation(sqq, ps_q, AF.Square)
                ps_nq = pps.tile([33, 512], FP, tag="pp")
                nc.tensor.matmul(ps_nq, ones_pat, sqq, start=True, stop=True)
                lnq = psb.tile([1, 512], FP, tag="ln")
                nc.scalar.activation(lnq, ps_nq[0:1, :], AF.Ln,
                                     bias=eps_b[0:1, :])
                rqb = psb.tile([1, 512], S_DT, tag="rqb")
                nc.scalar.activation(rqb, lnq, AF.Exp, scale=-0.5)
                lnq2 = psb.tile([1, 512], FP, tag="ln2")
                nc.scalar.activation(lnq2, ps_nq[32:33, :], AF.Ln,
                                     bias=eps_b[0:1, :])
                rkb = psb.tile([1, 512], S_DT, tag="rkb")
                nc.scalar.activation(rkb, lnq2, AF.Exp, scale=-0.5)

                # G' = (B^T X) * rq  and  X' = X * rk  (duplicated rows);
                # both normalizations fold into the S-matmul operands so the
                # main-loop exp needs no per-partition scale.
                ps_g = pps.tile([2 * C, 512], FP, tag="pp")
                nc.tensor.matmul(ps_g, Bsb2, Xs2[0:C, sl],
                                 start=True, stop=True)
                ps_rep = pps.tile([2 * C, 512], FP, tag="pp")
                nc.tensor.matmul(ps_rep, ones1_128, rqb,
                                 start=True, stop=True)
                rep_sb = psb.tile([2 * C, 512], FP, tag="rep")
                nc.vector.tensor_copy(rep_sb, ps_rep)
                nc.vector.tensor_mul(Gp2[:, sl], ps_g, rep_sb)
                ps_repk = pps.tile([2 * C, 512], FP, tag="pp")
                nc.tensor.matmul(ps_repk, ones1_128, rkb,
                                 start=True, stop=True)
                nc.vector.tensor_mul(Xp2[:, sl], ps_repk, Xs2[:, sl])

                # V' tiles for this chunk's 4 key ranges
                for j in range(4 * c8, 4 * c8 + 4):
                    ksl = slice(j * KC, j * KC + KC)
                    ps_kv = pps.tile([KC, HD], FP, tag="pp")
                    nc.tensor.matmul(ps_kv, X[:, ksl],
                                     WT[:, 2 * HD : 3 * HD],
                                     start=True, stop=True)
                    nc.vector.tensor_copy(Vp[:, j, 0:HD], ps_kv)

        # ---- main attention loop ----------------------------------------
        with contextlib.ExitStack() as mctx:
            ps_s_pool = mctx.enter_context(
                tc.tile_pool(name="ps_s", bufs=3, space="PSUM"))
            ps_o_pool = mctx.enter_context(
                tc.tile_pool(name="ps_o", bufs=2, space="PSUM"))
            pt_pool = mctx.enter_context(tc.tile_pool(name="pt", bufs=4))
            ep_pool = mctx.enter_context(tc.tile_pool(name="ep", bufs=2))
            def epilogue(nb, ps_o):
                # evacuate PSUM first (frees each ps_o bank), then divide by
                # the denominator row (partition 32), add residual, store.
                nbase = nb * NBQ
                oall = ep_pool.tile([33, NBQ], FP, tag="oall",
                                    name=f"oall_{nb}")
                for h2 in range(2):
                    nc.vector.tensor_copy(
                        oall[:, h2 * 512 : h2 * 512 + 512], ps_o[h2])
                lnd = ep_pool.tile([1, NBQ], FP, tag="lnd",
                                   name=f"lnd_{nb}")
                nc.scalar.activation(lnd, oall[32:33, :], AF.Ln)
                rinv = ep_pool.tile([1, NBQ], S_DT, tag="rinv",
                                    name=f"rinv_{nb}")
                nc.scalar.activation(rinv, lnd, AF.Exp, scale=-1.0)
                rep_sb = ep_pool.tile([HD, NBQ], FP, tag="repo",
                                      name=f"repo_{nb}")
                for h2 in range(2):
                    qsl = slice(h2 * 512, h2 * 512 + 512)
                    ps_rep = ps_s_pool.tile([HD, 512], FP, tag="ps_s",
                                            name=f"ps_rep_{nb}_{h2}")
                    nc.tensor.matmul(ps_rep, ones1_16, rinv[:, qsl],
                                     start=True, stop=True)
                    nc.vector.tensor_copy(rep_sb[:, qsl], ps_rep)
                t2 = ep_pool.tile([HD, NBQ], FP, tag="t2", name=f"t2_{nb}")
                nc.vector.tensor_mul(t2, oall[0:HD, :], rep_sb)
                osb = ep_pool.tile([HD, NBQ], FP, tag="osb",
                                   name=f"osb_{nb}")
                osl = slice(nbase, nbase + NBQ)
                nc.vector.tensor_add(osb, t2, XR[:, osl])
                nc.sync.dma_start(out_d[:, osl], osb)

            pending = None  # previous block's epilogue, deferred so the
            # next block's first S-matmuls/exps outrank it in priority
            for nb in range(NB):
                nbase = nb * NBQ
                ps_o = [ps_o_pool.tile([33, 512], FP, tag="ps_o",
                                       name=f"ps_o_{nb}_{h2}")
                        for h2 in range(2)]
                for j in range(JT):
                    ksl = slice(j * KC, j * KC + KC)
                    ps_s = ps_s_pool.tile([KC, NBQ], FP, tag="ps_s")
                    for h2 in range(2):
                        qsl = slice(h2 * 512, h2 * 512 + 512)
                        gsl = slice(nbase + h2 * 512, nbase + h2 * 512 + 512)
                        rg = slice(h2 * C, h2 * C + C)  # alternate row groups
                        nc.tensor.matmul(ps_s[:, qsl], Xp2[rg, ksl],
                                         Gp2[rg, gsl], start=True, stop=True)
                    pt = pt_pool.tile([KC, NBQ], PT_DT, tag="pt")
                    nc.scalar.activation(pt, ps_s, AF.Exp)
                    for h2 in range(2):
                        qsl = slice(h2 * 512, h2 * 512 + 512)
                        nc.tensor.matmul(ps_o[h2], Vp[:, j, :], pt[:, qsl],
                                         start=(j == 0), stop=(j == JT - 1))
                    if j == 2 and pending is not None:
                        epilogue(*pending)
                        pending = None
                pending = (nb, ps_o)
            epilogue(*pending)


_CACHE = {}


def _get_program():
    if "nc" not in _CACHE:
        _CACHE["nc"] = build_program()
    return _CACHE["nc"]


def make_in_maps(x, w_qkv):
    """Shard full inputs into per-core input maps. Core i = (b=i//4, h=i%4)."""
    x = np.ascontiguousarray(np.asarray(x, dtype=np.float32))
    w_qkv = np.ascontiguousarray(np.asarray(w_qkv, dtype=np.float32))
    b_, c, d, hh, ww = x.shape
    xf = x.reshape(b_, c, d * hh * ww)
    in_maps = []
    for core in range(NCORES):
        b, h = divmod(core, HEADS)
        rows = np.concatenate([
            np.arange(h * HD, (h + 1) * HD),
            np.arange(C + h * HD, C + (h + 1) * HD),
            np.arange(2 * C + h * HD, 2 * C + (h + 1) * HD),
        ])
        w_h = np.ascontiguousarray(w_qkv[rows, :])          # [48, 64]
        wT_h = np.ascontiguousarray(w_h.T)                   # [64, 48]
        x_b = np.ascontiguousarray(xf[b])                    # [64, 4096]
        x_res = np.ascontiguousarray(x_b[h * HD : (h + 1) * HD])  # [16, 4096]
        # col 0 sums q squares -> partition 0; col 32 sums k squares ->
        # partition 32 (PSUM reads must start 32-aligned)
        ones_pat = np.zeros((2 * HD, 33), dtype=np.float32)
        ones_pat[0:HD, 0] = 1.0
        ones_pat[HD : 2 * HD, 32] = 1.0
        in_maps.append({"x": x_b, "w": w_h, "wT": wT_h, "xres": x_res,
                        "onespat": ones_pat})
    return in_maps


def assemble_output(results, x_shape):
    b_, c, d, hh, ww = x_shape
    out = np.empty((b_, c, d * hh * ww), dtype=np.float32)
    for core in range(NCORES):
        b, h = divmod(core, HEADS)
        out[b, h * HD : (h + 1) * HD] = results[core]["out"]
    return out.reshape(x_shape)


def run(x, w_qkv, trace=False, **kw):
    nc = _get_program()
    in_maps = make_in_maps(x, w_qkv)
    res = run_bass_kernel_spmd(nc, in_maps, list(range(NCORES)),
                               trace=trace, **kw)
    return assemble_output(res.results, np.asarray(x).shape), res


def kernel(x, w_qkv):
    out, _ = run(x, w_qkv)
    return out


# revision 78
# speedup vs baseline: 1.0848x; 1.0848x over previous
"""Trainium2 Bass kernel for nn_Attention_40785009443452.

Reference computation (per batch b):
    qkv = w_qkv @ x_b            # 1x1x1 conv == channel linear
    q,k,v split into 4 heads of dim 16, tokens N = 16*16*16 = 4096
    q,k L2-normalized along head dim
    attn = softmax(q @ k^T)      # [N, N] per (b, head)
    out  = attn @ v  (+ x residual)

Sharding: 8 (batch, head) pairs -> 8 NeuronCores (data + head parallel).
Each core computes one full 4096x4096 attention.

Device algorithm (per core), S^T orientation so softmax reduction (over
keys) lands on the PSUM partition axis and is folded into the PV matmul
via an appended ones-column on V:

    B  = Wq^T Wk                     [64, 64]   (tiny matmul on device)
    G  = B^T X                       [64, 4096]
    G' = G * rq  (column scale)      rq[n] = 1/||q_n||
    S^T tile [128 keys, 1024 qry] = X_j^T(chunk) @ G'(cols)   K=64 matmul
    P^T = exp(rk[m] * S^T)           rk on ACT per-partition scale
    O' [17, 1024] += V'_j^T @ P^T    V' = [V_j | ones]  -> row 16 = denom
    out^T = O'[0:16] / O'[16] + x_res

All normalization scales are computed as exp(-0.5*ln(sumsq)) on ScalarE
(Rsqrt/Reciprocal activations are banned for accuracy; Ln+Exp live in one
ACT table set so there are no table switches).
"""

import numpy as np

import concourse.bass as bass
import concourse.mybir as mybir
import concourse.tile as tile
from concourse import bacc
from concourse.bass_utils import run_bass_kernel_spmd

NCORES = 8
C = 64          # channels
HEADS = 4
HD = 16         # head dim
N = 4096        # tokens (16*16*16)
NBQ = 1024      # queries per outer block
NB = N // NBQ   # 4 outer blocks
KC = 128        # keys per chunk
JT = N // KC    # 32 key chunks
FP = mybir.dt.float32

# dtype for the P = exp(S) tiles and V' (the PV matmul operands)
PT_DT = mybir.dt.bfloat16
# dtype for the S^T matmul operands (X stationary copy + G' moving)
S_DT = mybir.dt.bfloat16

AF = mybir.ActivationFunctionType


def build_program():
    nc = bacc.Bacc(
        "TRN2", target_bir_lowering=False, debug=False, enable_asserts=False
    )
    x_d = nc.dram_tensor("x", [C, N], FP, kind="ExternalInput").ap()
    w_d = nc.dram_tensor("w", [3 * HD, C], FP, kind="ExternalInput").ap()
    wT_d = nc.dram_tensor("wT", [C, 3 * HD], FP, kind="ExternalInput").ap()
    xr_d = nc.dram_tensor("xres", [HD, N], FP, kind="ExternalInput").ap()
    op_d = nc.dram_tensor("onespat", [2 * HD, 33], FP,
                          kind="ExternalInput").ap()
    out_d = nc.dram_tensor("out", [HD, N], FP, kind="ExternalOutput").ap()
    scr_d = nc.dram_tensor("rk_scratch", [1, N], FP, kind="Internal").ap()

    with tile.TileContext(nc) as tc:
        _body(tc, x_d, w_d, wT_d, xr_d, op_d, out_d, scr_d)
    nc.compile()
    return nc


def _body(tc, x_d, w_d, wT_d, xr_d, op_d, out_d, scr_d):
    nc = tc.nc
    import contextlib

    import os

    # Pre-load the one ACT table set that contains Exp, Ln AND Square, so the
    # compiler's per-function chooser doesn't flip-flop between
    # exp_and_others and natural_log (35 table loads = ~45us of ACT time).
    if os.environ.get("K_PRELOAD", "1") == "1":
        from concourse.hw_specs import get_activation_tables

        set_names = list(get_activation_tables(nc.m.arch).keys())
        set_id = set_names.index("natural_log_exp_and_others")
        nc.scalar.add_instruction(
            mybir.InstLoadActFuncSet(
                name=f"I-{nc.next_id()}", act_func_set_id=set_id
            )
        )

    with contextlib.ExitStack() as ctx:
        consts = ctx.enter_context(tc.tile_pool(name="consts", bufs=1))

        # ---- load inputs -------------------------------------------------
        # weights on a separate DMA queue (small, needed first); x chunks on
        # the sync queue so compute starts as soon as chunk 0 lands.
        wq_eng = nc.gpsimd if os.environ.get("K_GPDMA", "1") == "1" else nc.sync
        Wq = consts.tile([HD, C], FP)
        wq_eng.dma_start(Wq, w_d[0:HD, :])
        Wk = consts.tile([HD, C], FP)
        wq_eng.dma_start(Wk, w_d[HD : 2 * HD, :])
        WT = consts.tile([C, 3 * HD], FP)
        wq_eng.dma_start(WT, wT_d)
        X = consts.tile([C, N], FP)
        for c8 in range(8):
            sl = slice(c8 * 512, c8 * 512 + 512)
            nc.sync.dma_start(X[:, sl], x_d[:, sl])
        XR = consts.tile([HD, N], FP)
        wq_eng.dma_start(XR, xr_d)

        ones1_16 = consts.tile([1, HD], S_DT)
        nc.any.memset(ones1_16, 1.0)
        eps_b = consts.tile([KC, 1], FP)
        nc.any.memset(eps_b, 1e-24)

        # Duplicated-row (both halves identical) bf16 operands: the two
        # 512-column S matmuls of each key chunk run on PE row groups 0-63
        # and 64-127 — alternating row groups lets the PE pull LDWEIGHTS
        # ahead and run the K=64 matmuls concurrently (2.2x measured).
        Bsb2 = consts.tile([C, 2 * C], S_DT)   # [B | B] stationary
        Gp2 = consts.tile([2 * C, N], S_DT)    # G'*rq duplicated rows
        Xs2 = consts.tile([2 * C, N], S_DT)    # X duplicated rows
        Xp2 = consts.tile([2 * C, N], S_DT)    # X*rk duplicated rows
        WTb = consts.tile([C, 2 * HD], S_DT)   # [Wq^T | Wk^T] in bf16
        ones_pat_f = consts.tile([2 * HD, 33], FP)
        nc.sync.dma_start(ones_pat_f, op_d)
        ones_pat = consts.tile([2 * HD, 33], S_DT)
        nc.vector.tensor_copy(ones_pat, ones_pat_f)
        ones1_128 = consts.tile([1, 2 * C], S_DT)
        nc.any.memset(ones1_128, 1.0)
        # [V_j(16) | zeros(16) | ones(1)] stationary tiles; the ones column
        # lands the softmax denominator on PSUM partition 32 (32-aligned
        # reads are a BIR verifier requirement).
        Vp = consts.tile([KC, JT, 33], PT_DT)

        nc.any.memset(Vp, 0.0)
        nc.any.memset(Vp[:, :, 32], 1.0)

        with contextlib.ExitStack() as mctx:
            pps = mctx.enter_context(
                tc.tile_pool(name="prol_ps", bufs=8, space="PSUM"))
            psb = mctx.enter_context(tc.tile_pool(name="prol_sb", bufs=6))

            # B = Wq^T Wk (tiny, fp32), duplicated into [B | B] bf16
            ps_b = pps.tile([C, C], FP, tag="pp", bufs=3)
            nc.tensor.matmul(ps_b, Wq, Wk, start=True, stop=True)
            nc.vector.tensor_copy(Bsb2[:, 0:C], ps_b)
            nc.vector.tensor_copy(Bsb2[:, C : 2 * C], ps_b)
            nc.vector.tensor_copy(WTb, WT[:, 0 : 2 * HD])

            # Per 512-column chunk: q norms + k norms/V tiles (interleaved so
            # PE always has independent work while ACT runs Ln/Exp).
            for c4 in range(4):
                sl = slice(c4 * 1024, c4 * 1024 + 1024)
                nc.vector.tensor_copy(Xs2[0:C, sl], X[:, sl])
                nc.vector.tensor_copy(Xs2[C : 2 * C, sl], X[:, sl])
                # q + k norms (orientation 1): sumsq_q -> partition 0,
                # sumsq_k -> partition 32 of ps_nq
                ps_q = pps.tile([2 * HD, 1024], FP, tag="pp", bufs=3)
                sqq = psb.tile([2 * HD, 1024], S_DT, tag="sq")
                ps_nq = pps.tile([33, 1024], FP, tag="pp", bufs=3)
                for h2 in range(2):
                    hsl = slice(h2 * 512, h2 * 512 + 512)
                    xsl = slice(c4 * 1024 + h2 * 512,
                                c4 * 1024 + h2 * 512 + 512)
                    nc.tensor.matmul(ps_q[:, hsl], WTb, Xs2[0:C, xsl],
                                     start=True, stop=True)
                nc.scalar.activation(sqq, ps_q, AF.Square)
                for h2 in range(2):
                    hsl = slice(h2 * 512, h2 * 512 + 512)
                    nc.tensor.matmul(ps_nq[:, hsl], ones_pat, sqq[:, hsl],
                                     start=True, stop=True)
                lnq = psb.tile([1, 1024], FP, tag="ln")
                nc.scalar.activation(lnq, ps_nq[0:1, :], AF.Ln,
                                     bias=eps_b[0:1, :])
                rqb = psb.tile([1, 1024], S_DT, tag="rqb")
                nc.scalar.activation(rqb, lnq, AF.Exp, scale=-0.5)
                lnq2 = psb.tile([1, 1024], FP, tag="ln2")
                nc.scalar.activation(lnq2, ps_nq[32:33, :], AF.Ln,
                                     bias=eps_b[0:1, :])
                rkb = psb.tile([1, 1024], S_DT, tag="rkb")
                nc.scalar.activation(rkb, lnq2, AF.Exp, scale=-0.5)

                # G' = (B^T X) * rq  and  X' = X * rk  (duplicated rows);
                # both normalizations fold into the S-matmul operands so the
                # main-loop exp needs no per-partition scale.
                ps_g = pps.tile([2 * C, 1024], FP, tag="pp", bufs=3)
                ps_rep = pps.tile([2 * C, 1024], FP, tag="pp", bufs=3)
                ps_repk = pps.tile([2 * C, 1024], FP, tag="pp", bufs=3)
                for h2 in range(2):
                    hsl = slice(h2 * 512, h2 * 512 + 512)
                    xsl = slice(c4 * 1024 + h2 * 512,
                                c4 * 1024 + h2 * 512 + 512)
                    nc.tensor.matmul(ps_g[:, hsl], Bsb2, Xs2[0:C, xsl],
                                     start=True, stop=True)
                    nc.tensor.matmul(ps_rep[:, hsl], ones1_128,
                                     rqb[:, hsl], start=True, stop=True)
                    nc.tensor.matmul(ps_repk[:, hsl], ones1_128,
                                     rkb[:, hsl], start=True, stop=True)
                rep_sb = psb.tile([2 * C, 1024], FP, tag="rep")
                nc.vector.tensor_copy(rep_sb, ps_rep)
                nc.vector.tensor_mul(Gp2[:, sl], ps_g, rep_sb)
                nc.vector.tensor_mul(Xp2[:, sl], ps_repk, Xs2[:, sl])

                # V' tiles for this chunk's 8 key ranges
                for j in range(8 * c4, 8 * c4 + 8):
                    ksl = slice(j * KC, j * KC + KC)
                    ps_kv = pps.tile([KC, HD], FP, tag="ppv", bufs=2)
                    nc.tensor.matmul(ps_kv, X[:, ksl],
                                     WT[:, 2 * HD : 3 * HD],
                                     start=True, stop=True)
                    nc.vector.tensor_copy(Vp[:, j, 0:HD], ps_kv)

        # ---- main attention loop ----------------------------------------
        with contextlib.ExitStack() as mctx:
            ps_s_pool = mctx.enter_context(
                tc.tile_pool(name="ps_s", bufs=3, space="PSUM"))
            ps_o_pool = mctx.enter_context(
                tc.tile_pool(name="ps_o", bufs=2, space="PSUM"))
            pt_pool = mctx.enter_context(tc.tile_pool(name="pt", bufs=4))
            ep_pool = mctx.enter_context(tc.tile_pool(name="ep", bufs=2))
            def epilogue(nb, ps_o):
                # evacuate PSUM first (frees each ps_o bank), then divide by
                # the denominator row (partition 32), add residual, store.
                nbase = nb * NBQ
                oall = ep_pool.tile([33, NBQ], FP, tag="oall",
                                    name=f"oall_{nb}")
                for h2 in range(2):
                    nc.vector.tensor_copy(
                        oall[:, h2 * 512 : h2 * 512 + 512], ps_o[h2])
                lnd = ep_pool.tile([1, NBQ], FP, tag="lnd",
                                   name=f"lnd_{nb}")
                nc.scalar.activation(lnd, oall[32:33, :], AF.Ln)
                rinv = ep_pool.tile([1, NBQ], S_DT, tag="rinv",
                                    name=f"rinv_{nb}")
                nc.scalar.activation(rinv, lnd, AF.Exp, scale=-1.0)
                rep_sb = ep_pool.tile([HD, NBQ], FP, tag="repo",
                                      name=f"repo_{nb}")
                for h2 in range(2):
                    qsl = slice(h2 * 512, h2 * 512 + 512)
                    ps_rep = ps_s_pool.tile([HD, 512], FP, tag="ps_s",
                                            name=f"ps_rep_{nb}_{h2}")
                    nc.tensor.matmul(ps_rep, ones1_16, rinv[:, qsl],
                                     start=True, stop=True)
                    nc.vector.tensor_copy(rep_sb[:, qsl], ps_rep)
                t2 = ep_pool.tile([HD, NBQ], FP, tag="t2", name=f"t2_{nb}")
                nc.vector.tensor_mul(t2, oall[0:HD, :], rep_sb)
                osb = ep_pool.tile([HD, NBQ], FP, tag="osb",
                                   name=f"osb_{nb}")
                osl = slice(nbase, nbase + NBQ)
                nc.vector.tensor_add(osb, t2, XR[:, osl])
                nc.sync.dma_start(out_d[:, osl], osb)

            pending = None  # previous block's epilogue, deferred so the
            # next block's first S-matmuls/exps outrank it in priority
            for nb in range(NB):
                nbase = nb * NBQ
                ps_o = [ps_o_pool.tile([33, 512], FP, tag="ps_o",
                                       name=f"ps_o_{nb}_{h2}")
                        for h2 in range(2)]
                for j in range(JT):
                    ksl = slice(j * KC, j * KC + KC)
                    ps_s = ps_s_pool.tile([KC, NBQ], FP, tag="ps_s")
                    for h2 in range(2):
                        qsl = slice(h2 * 512, h2 * 512 + 512)
                        gsl = slice(nbase + h2 * 512, nbase + h2 * 512 + 512)
                        rg = slice(h2 * C, h2 * C + C)  # alternate row groups
                        nc.tensor.matmul(ps_s[:, qsl], Xp2[rg, ksl],
                                         Gp2[rg, gsl], start=True, stop=True)
                    pt = pt_pool.tile([KC, NBQ], PT_DT, tag="pt")
                    nc.scalar.activation(pt, ps_s, AF.Exp)
                    for h2 in range(2):
                        qsl = slice(h2 * 512, h2 * 512 + 512)
                        nc.tensor.matmul(ps_o[h2], Vp[:, j, :], pt[:, qsl],
                                         start=(j == 0), stop=(j == JT - 1))
                    if j == 2 and pending is not None:
                        epilogue(*pending)
                        pending = None
                pending = (nb, ps_o)
            epilogue(*pending)


_CACHE = {}


def _get_program():
    if "nc" not in _CACHE:
        _CACHE["nc"] = build_program()
    return _CACHE["nc"]


def make_in_maps(x, w_qkv):
    """Shard full inputs into per-core input maps. Core i = (b=i//4, h=i%4)."""
    x = np.ascontiguousarray(np.asarray(x, dtype=np.float32))
    w_qkv = np.ascontiguousarray(np.asarray(w_qkv, dtype=np.float32))
    b_, c, d, hh, ww = x.shape
    xf = x.reshape(b_, c, d * hh * ww)
    in_maps = []
    for core in range(NCORES):
        b, h = divmod(core, HEADS)
        rows = np.concatenate([
            np.arange(h * HD, (h + 1) * HD),
            np.arange(C + h * HD, C + (h + 1) * HD),
            np.arange(2 * C + h * HD, 2 * C + (h + 1) * HD),
        ])
        w_h = np.ascontiguousarray(w_qkv[rows, :])          # [48, 64]
        wT_h = np.ascontiguousarray(w_h.T)                   # [64, 48]
        x_b = np.ascontiguousarray(xf[b])                    # [64, 4096]
        x_res = np.ascontiguousarray(x_b[h * HD : (h + 1) * HD])  # [16, 4096]
        # col 0 sums q squares -> partition 0; col 32 sums k squares ->
        # partition 32 (PSUM reads must start 32-aligned)
        ones_pat = np.zeros((2 * HD, 33), dtype=np.float32)
        ones_pat[0:HD, 0] = 1.0
        ones_pat[HD : 2 * HD, 32] = 1.0
        in_maps.append({"x": x_b, "w": w_h, "wT": wT_h, "xres": x_res,
                        "onespat": ones_pat})
    return in_maps


def assemble_output(results, x_shape):
    b_, c, d, hh, ww = x_shape
    out = np.empty((b_, c, d * hh * ww), dtype=np.float32)
    for core in range(NCORES):
        b, h = divmod(core, HEADS)
        out[b, h * HD : (h + 1) * HD] = results[core]["out"]
    return out.reshape(x_shape)


def run(x, w_qkv, trace=False, **kw):
    nc = _get_program()
    in_maps = make_in_maps(x, w_qkv)
    res = run_bass_kernel_spmd(nc, in_maps, list(range(NCORES)),
                               trace=trace, **kw)
    return assemble_output(res.results, np.asarray(x).shape), res


def kernel(x, w_qkv):
    out, _ = run(x, w_qkv)
    return out


# revision 79
# speedup vs baseline: 1.0908x; 1.0056x over previous
"""Trainium2 Bass kernel for nn_Attention_40785009443452.

Reference computation (per batch b):
    qkv = w_qkv @ x_b            # 1x1x1 conv == channel linear
    q,k,v split into 4 heads of dim 16, tokens N = 16*16*16 = 4096
    q,k L2-normalized along head dim
    attn = softmax(q @ k^T)      # [N, N] per (b, head)
    out  = attn @ v  (+ x residual)

Sharding: 8 (batch, head) pairs -> 8 NeuronCores (data + head parallel).
Each core computes one full 4096x4096 attention.

Device algorithm (per core), S^T orientation so softmax reduction (over
keys) lands on the PSUM partition axis and is folded into the PV matmul
via an appended ones-column on V:

    B  = Wq^T Wk                     [64, 64]   (tiny matmul on device)
    G  = B^T X                       [64, 4096]
    G' = G * rq  (column scale)      rq[n] = 1/||q_n||
    S^T tile [128 keys, 1024 qry] = X_j^T(chunk) @ G'(cols)   K=64 matmul
    P^T = exp(rk[m] * S^T)           rk on ACT per-partition scale
    O' [17, 1024] += V'_j^T @ P^T    V' = [V_j | ones]  -> row 16 = denom
    out^T = O'[0:16] / O'[16] + x_res

All normalization scales are computed as exp(-0.5*ln(sumsq)) on ScalarE
(Rsqrt/Reciprocal activations are banned for accuracy; Ln+Exp live in one
ACT table set so there are no table switches).
"""

import numpy as np

import concourse.bass as bass
import concourse.mybir as mybir
import concourse.tile as tile
from concourse import bacc
from concourse.bass_utils import run_bass_kernel_spmd

NCORES = 8
C = 64          # channels
HEADS = 4
HD = 16         # head dim
N = 4096        # tokens (16*16*16)
NBQ = 1024      # queries per outer block
NB = N // NBQ   # 4 outer blocks
KC = 128        # keys per chunk
JT = N // KC    # 32 key chunks
FP = mybir.dt.float32

# dtype for the P = exp(S) tiles and V' (the PV matmul operands)
PT_DT = mybir.dt.bfloat16
# dtype for the S^T matmul operands (X stationary copy + G' moving)
S_DT = mybir.dt.bfloat16

AF = mybir.ActivationFunctionType


def build_program():
    nc = bacc.Bacc(
        "TRN2", target_bir_lowering=False, debug=False, enable_asserts=False
    )
    x_d = nc.dram_tensor("x", [C, N], FP, kind="ExternalInput").ap()
    w_d = nc.dram_tensor("w", [3 * HD, C], FP, kind="ExternalInput").ap()
    wT_d = nc.dram_tensor("wT", [C, 3 * HD], FP, kind="ExternalInput").ap()
    xr_d = nc.dram_tensor("xres", [HD, N], FP, kind="ExternalInput").ap()
    op_d = nc.dram_tensor("onespat", [2 * HD, 33], FP,
                          kind="ExternalInput").ap()
    out_d = nc.dram_tensor("out", [HD, N], FP, kind="ExternalOutput").ap()
    scr_d = nc.dram_tensor("rk_scratch", [1, N], FP, kind="Internal").ap()

    with tile.TileContext(nc) as tc:
        _body(tc, x_d, w_d, wT_d, xr_d, op_d, out_d, scr_d)
    nc.compile()
    return nc


def _body(tc, x_d, w_d, wT_d, xr_d, op_d, out_d, scr_d):
    nc = tc.nc
    import contextlib

    import os

    # Pre-load the one ACT table set that contains Exp, Ln AND Square, so the
    # compiler's per-function chooser doesn't flip-flop between
    # exp_and_others and natural_log (35 table loads = ~45us of ACT time).
    if os.environ.get("K_PRELOAD", "1") == "1":
        from concourse.hw_specs import get_activation_tables

        set_names = list(get_activation_tables(nc.m.arch).keys())
        set_id = set_names.index("natural_log_exp_and_others")
        nc.scalar.add_instruction(
            mybir.InstLoadActFuncSet(
                name=f"I-{nc.next_id()}", act_func_set_id=set_id
            )
        )

    with contextlib.ExitStack() as ctx:
        consts = ctx.enter_context(tc.tile_pool(name="consts", bufs=1))

        # ---- load inputs -------------------------------------------------
        # weights on a separate DMA queue (small, needed first); x chunks on
        # the sync queue so compute starts as soon as chunk 0 lands.
        wq_eng = nc.gpsimd if os.environ.get("K_GPDMA", "1") == "1" else nc.sync
        Wq = consts.tile([HD, C], FP)
        wq_eng.dma_start(Wq, w_d[0:HD, :])
        Wk = consts.tile([HD, C], FP)
        wq_eng.dma_start(Wk, w_d[HD : 2 * HD, :])
        WT = consts.tile([C, 3 * HD], FP)
        wq_eng.dma_start(WT, wT_d)
        X = consts.tile([C, N], FP)
        for c8 in range(8):
            sl = slice(c8 * 512, c8 * 512 + 512)
            nc.sync.dma_start(X[:, sl], x_d[:, sl])
        XR = consts.tile([HD, N], FP)
        wq_eng.dma_start(XR, xr_d)

        ones1_16 = consts.tile([1, HD], S_DT)
        nc.any.memset(ones1_16, 1.0)
        eps_b = consts.tile([KC, 1], FP)
        nc.any.memset(eps_b, 1e-24)

        # Duplicated-row (both halves identical) bf16 operands: the two
        # 512-column S matmuls of each key chunk run on PE row groups 0-63
        # and 64-127 — alternating row groups lets the PE pull LDWEIGHTS
        # ahead and run the K=64 matmuls concurrently (2.2x measured).
        Bsb2 = consts.tile([C, 2 * C], S_DT)   # [B | B] stationary
        Gp2 = consts.tile([2 * C, N], S_DT)    # G'*rq duplicated rows
        Xs2 = consts.tile([2 * C, N], S_DT)    # X duplicated rows
        Xp2 = consts.tile([2 * C, N], S_DT)    # X*rk duplicated rows
        WTb = consts.tile([C, 2 * HD], S_DT)   # [Wq^T | Wk^T] in bf16
        ones_pat_f = consts.tile([2 * HD, 33], FP)
        nc.sync.dma_start(ones_pat_f, op_d)
        ones_pat = consts.tile([2 * HD, 33], S_DT)
        nc.vector.tensor_copy(ones_pat, ones_pat_f)
        ones1_128 = consts.tile([1, 2 * C], S_DT)
        nc.any.memset(ones1_128, 1.0)
        # [V_j(16) | zeros(16) | ones(1)] stationary tiles; the ones column
        # lands the softmax denominator on PSUM partition 32 (32-aligned
        # reads are a BIR verifier requirement).
        Vp = consts.tile([KC, JT, 33], PT_DT)

        nc.any.memset(Vp, 0.0)
        nc.any.memset(Vp[:, :, 32], 1.0)

        with contextlib.ExitStack() as mctx:
            pps = mctx.enter_context(
                tc.tile_pool(name="prol_ps", bufs=8, space="PSUM"))
            psb = mctx.enter_context(tc.tile_pool(name="prol_sb", bufs=6))

            # B = Wq^T Wk (tiny, fp32), duplicated into [B | B] bf16
            ps_b = pps.tile([C, C], FP, tag="pp", bufs=3)
            nc.tensor.matmul(ps_b, Wq, Wk, start=True, stop=True)
            nc.vector.tensor_copy(Bsb2[:, 0:C], ps_b)
            nc.vector.tensor_copy(Bsb2[:, C : 2 * C], ps_b)
            nc.vector.tensor_copy(WTb, WT[:, 0 : 2 * HD])

            # Per 512-column chunk: q norms + k norms/V tiles (interleaved so
            # PE always has independent work while ACT runs Ln/Exp).
            for c4 in range(4):
                sl = slice(c4 * 1024, c4 * 1024 + 1024)

                # V' tiles for this chunk's 8 key ranges
                for j in range(8 * c4, 8 * c4 + 8):
                    ksl = slice(j * KC, j * KC + KC)
                    ps_kv = pps.tile([KC, HD], FP, tag="ppv", bufs=2)
                    nc.tensor.matmul(ps_kv, X[:, ksl],
                                     WT[:, 2 * HD : 3 * HD],
                                     start=True, stop=True)
                    nc.vector.tensor_copy(Vp[:, j, 0:HD], ps_kv)
                nc.vector.tensor_copy(Xs2[0:C, sl], X[:, sl])
                nc.vector.tensor_copy(Xs2[C : 2 * C, sl], X[:, sl])
                # q + k norms (orientation 1): sumsq_q -> partition 0,
                # sumsq_k -> partition 32 of ps_nq
                ps_q = pps.tile([2 * HD, 1024], FP, tag="pp", bufs=3)
                sqq = psb.tile([2 * HD, 1024], S_DT, tag="sq")
                ps_nq = pps.tile([33, 1024], FP, tag="pp", bufs=3)
                for h2 in range(2):
                    hsl = slice(h2 * 512, h2 * 512 + 512)
                    xsl = slice(c4 * 1024 + h2 * 512,
                                c4 * 1024 + h2 * 512 + 512)
                    nc.tensor.matmul(ps_q[:, hsl], WTb, Xs2[0:C, xsl],
                                     start=True, stop=True)
                nc.scalar.activation(sqq, ps_q, AF.Square)
                for h2 in range(2):
                    hsl = slice(h2 * 512, h2 * 512 + 512)
                    nc.tensor.matmul(ps_nq[:, hsl], ones_pat, sqq[:, hsl],
                                     start=True, stop=True)
                lnq = psb.tile([1, 1024], FP, tag="ln")
                nc.scalar.activation(lnq, ps_nq[0:1, :], AF.Ln,
                                     bias=eps_b[0:1, :])
                rqb = psb.tile([1, 1024], S_DT, tag="rqb")
                nc.scalar.activation(rqb, lnq, AF.Exp, scale=-0.5)
                lnq2 = psb.tile([1, 1024], FP, tag="ln2")
                nc.scalar.activation(lnq2, ps_nq[32:33, :], AF.Ln,
                                     bias=eps_b[0:1, :])
                rkb = psb.tile([1, 1024], S_DT, tag="rkb")
                nc.scalar.activation(rkb, lnq2, AF.Exp, scale=-0.5)

                # G' = (B^T X) * rq  and  X' = X * rk  (duplicated rows);
                # both normalizations fold into the S-matmul operands so the
                # main-loop exp needs no per-partition scale.
                ps_g = pps.tile([2 * C, 1024], FP, tag="pp", bufs=3)
                ps_rep = pps.tile([2 * C, 1024], FP, tag="pp", bufs=3)
                ps_repk = pps.tile([2 * C, 1024], FP, tag="pp", bufs=3)
                for h2 in range(2):
                    hsl = slice(h2 * 512, h2 * 512 + 512)
                    xsl = slice(c4 * 1024 + h2 * 512,
                                c4 * 1024 + h2 * 512 + 512)
                    nc.tensor.matmul(ps_g[:, hsl], Bsb2, Xs2[0:C, xsl],
                                     start=True, stop=True)
                    nc.tensor.matmul(ps_rep[:, hsl], ones1_128,
                                     rqb[:, hsl], start=True, stop=True)
                    nc.tensor.matmul(ps_repk[:, hsl], ones1_128,
                                     rkb[:, hsl], start=True, stop=True)
                rep_sb = psb.tile([2 * C, 1024], FP, tag="rep")
                nc.vector.tensor_copy(rep_sb, ps_rep)
                nc.vector.tensor_mul(Gp2[:, sl], ps_g, rep_sb)
                nc.vector.tensor_mul(Xp2[:, sl], ps_repk, Xs2[:, sl])


        # ---- main attention loop ----------------------------------------
        with contextlib.ExitStack() as mctx:
            ps_s_pool = mctx.enter_context(
                tc.tile_pool(name="ps_s", bufs=3, space="PSUM"))
            ps_o_pool = mctx.enter_context(
                tc.tile_pool(name="ps_o", bufs=2, space="PSUM"))
            pt_pool = mctx.enter_context(tc.tile_pool(name="pt", bufs=4))
            ep_pool = mctx.enter_context(tc.tile_pool(name="ep", bufs=2))
            def epilogue(nb, ps_o):
                # evacuate PSUM first (frees each ps_o bank), then divide by
                # the denominator row (partition 32), add residual, store.
                nbase = nb * NBQ
                oall = ep_pool.tile([33, NBQ], FP, tag="oall",
                                    name=f"oall_{nb}")
                for h2 in range(2):
                    nc.vector.tensor_copy(
                        oall[:, h2 * 512 : h2 * 512 + 512], ps_o[h2])
                lnd = ep_pool.tile([1, NBQ], FP, tag="lnd",
                                   name=f"lnd_{nb}")
                nc.scalar.activation(lnd, oall[32:33, :], AF.Ln)
                rinv = ep_pool.tile([1, NBQ], S_DT, tag="rinv",
                                    name=f"rinv_{nb}")
                nc.scalar.activation(rinv, lnd, AF.Exp, scale=-1.0)
                rep_sb = ep_pool.tile([HD, NBQ], FP, tag="repo",
                                      name=f"repo_{nb}")
                for h2 in range(2):
                    qsl = slice(h2 * 512, h2 * 512 + 512)
                    ps_rep = ps_s_pool.tile([HD, 512], FP, tag="ps_s",
                                            name=f"ps_rep_{nb}_{h2}")
                    nc.tensor.matmul(ps_rep, ones1_16, rinv[:, qsl],
                                     start=True, stop=True)
                    nc.vector.tensor_copy(rep_sb[:, qsl], ps_rep)
                t2 = ep_pool.tile([HD, NBQ], FP, tag="t2", name=f"t2_{nb}")
                nc.vector.tensor_mul(t2, oall[0:HD, :], rep_sb)
                osb = ep_pool.tile([HD, NBQ], FP, tag="osb",
                                   name=f"osb_{nb}")
                osl = slice(nbase, nbase + NBQ)
                nc.vector.tensor_add(osb, t2, XR[:, osl])
                nc.sync.dma_start(out_d[:, osl], osb)

            pending = None  # previous block's epilogue, deferred so the
            # next block's first S-matmuls/exps outrank it in priority
            for nb in range(NB):
                nbase = nb * NBQ
                ps_o = [ps_o_pool.tile([33, 512], FP, tag="ps_o",
                                       name=f"ps_o_{nb}_{h2}")
                        for h2 in range(2)]
                for j in range(JT):
                    ksl = slice(j * KC, j * KC + KC)
                    ps_s = ps_s_pool.tile([KC, NBQ], FP, tag="ps_s")
                    for h2 in range(2):
                        qsl = slice(h2 * 512, h2 * 512 + 512)
                        gsl = slice(nbase + h2 * 512, nbase + h2 * 512 + 512)
                        rg = slice(h2 * C, h2 * C + C)  # alternate row groups
                        nc.tensor.matmul(ps_s[:, qsl], Xp2[rg, ksl],
                                         Gp2[rg, gsl], start=True, stop=True)
                    pt = pt_pool.tile([KC, NBQ], PT_DT, tag="pt")
                    nc.scalar.activation(pt, ps_s, AF.Exp)
                    for h2 in range(2):
                        qsl = slice(h2 * 512, h2 * 512 + 512)
                        nc.tensor.matmul(ps_o[h2], Vp[:, j, :], pt[:, qsl],
                                         start=(j == 0), stop=(j == JT - 1))
                    if j == 2 and pending is not None:
                        epilogue(*pending)
                        pending = None
                pending = (nb, ps_o)
            epilogue(*pending)


_CACHE = {}


def _get_program():
    if "nc" not in _CACHE:
        _CACHE["nc"] = build_program()
    return _CACHE["nc"]


def make_in_maps(x, w_qkv):
    """Shard full inputs into per-core input maps. Core i = (b=i//4, h=i%4)."""
    x = np.ascontiguousarray(np.asarray(x, dtype=np.float32))
    w_qkv = np.ascontiguousarray(np.asarray(w_qkv, dtype=np.float32))
    b_, c, d, hh, ww = x.shape
    xf = x.reshape(b_, c, d * hh * ww)
    in_maps = []
    for core in range(NCORES):
        b, h = divmod(core, HEADS)
        rows = np.concatenate([
            np.arange(h * HD, (h + 1) * HD),
            np.arange(C + h * HD, C + (h + 1) * HD),
            np.arange(2 * C + h * HD, 2 * C + (h + 1) * HD),
        ])
        w_h = np.ascontiguousarray(w_qkv[rows, :])          # [48, 64]
        wT_h = np.ascontiguousarray(w_h.T)                   # [64, 48]
        x_b = np.ascontiguousarray(xf[b])                    # [64, 4096]
        x_res = np.ascontiguousarray(x_b[h * HD : (h + 1) * HD])  # [16, 4096]
        # col 0 sums q squares -> partition 0; col 32 sums k squares ->
        # partition 32 (PSUM reads must start 32-aligned)
        ones_pat = np.zeros((2 * HD, 33), dtype=np.float32)
        ones_pat[0:HD, 0] = 1.0
        ones_pat[HD : 2 * HD, 32] = 1.0
        in_maps.append({"x": x_b, "w": w_h, "wT": wT_h, "xres": x_res,
                        "onespat": ones_pat})
    return in_maps


def assemble_output(results, x_shape):
    b_, c, d, hh, ww = x_shape
    out = np.empty((b_, c, d * hh * ww), dtype=np.float32)
    for core in range(NCORES):
        b, h = divmod(core, HEADS)
        out[b, h * HD : (h + 1) * HD] = results[core]["out"]
    return out.reshape(x_shape)


def run(x, w_qkv, trace=False, **kw):
    nc = _get_program()
    in_maps = make_in_maps(x, w_qkv)
    res = run_bass_kernel_spmd(nc, in_maps, list(range(NCORES)),
                               trace=trace, **kw)
    return assemble_output(res.results, np.asarray(x).shape), res


def kernel(x, w_qkv):
    out, _ = run(x, w_qkv)
    return out
